# revision 1
# baseline (speedup 1.0000x reference)
"""BatchTaskAlignedAssigner on 8 NeuronCores, pure data-parallel over batch.

Math is an exact reformulation of the reference that avoids top_k/scatter/
gather (reliably compiled on the neuron backend):
  - top-13 selection -> 13th-largest threshold via iterative max extraction
    (all in-gt-box priors have strictly positive metric, so
    pos = (metric >= thr) & (metric > 0) reproduces is_in_topk * mask_in_gts)
  - per-(b,g) class gather  -> one-hot einsum (exact: adds only zeros)
  - one_hot(argmax overlaps) -> equality-with-colmax (used only where >1 gt
    matched, where overlaps are continuous and untied)
  - target gathers by assigned gt -> einsum against the 0/1 pos mask with a
    row-0 fixup for background columns.
"""
import numpy as np
import jax
import jax.numpy as jnp

TOPK = 13
ALPHA = 1.0
BETA = 6.0
EPS = 1e-9
IOU_EPS = 1e-6
NCORES = 8
B, P, C, G = 32, 8400, 80, 64


def _pairwise_iou(gt_bboxes, pred_bboxes):
    lt = jnp.maximum(gt_bboxes[:, :, None, :2], pred_bboxes[:, None, :, :2])
    rb = jnp.minimum(gt_bboxes[:, :, None, 2:], pred_bboxes[:, None, :, 2:])
    wh = jnp.clip(rb - lt, 0.0)
    inter = wh[..., 0] * wh[..., 1]
    area_g = (gt_bboxes[..., 2] - gt_bboxes[..., 0]) * (gt_bboxes[..., 3] - gt_bboxes[..., 1])
    area_p = (pred_bboxes[..., 2] - pred_bboxes[..., 0]) * (pred_bboxes[..., 3] - pred_bboxes[..., 1])
    union = area_g[:, :, None] + area_p[:, None, :] - inter
    return inter / jnp.maximum(union, IOU_EPS)


def _assign(pred_scores, pred_bboxes, priors_points, gt_labels, gt_bboxes, pad_bbox_flag):
    labels = gt_labels[..., 0].astype(jnp.int32)  # [b,G]
    oh = jax.nn.one_hot(labels, C, dtype=pred_scores.dtype)          # [b,G,C]
    bbox_scores = jnp.einsum('bgc,bpc->bgp', oh, pred_scores)        # [b,G,P]
    overlaps = _pairwise_iou(gt_bboxes, pred_bboxes)                 # [b,G,P]
    align = (bbox_scores ** ALPHA) * (overlaps ** BETA)

    deltas = jnp.concatenate([
        priors_points[None, None] - gt_bboxes[:, :, None, :2],
        gt_bboxes[:, :, None, 2:] - priors_points[None, None]], axis=-1)
    mask_in_gts = (deltas.min(axis=-1) > EPS).astype(align.dtype)
    metrics = align * mask_in_gts

    # 13th-largest value per (b,g) row: extract the max 12 times, then max.
    t = metrics
    for _ in range(TOPK - 1):
        m = t.max(axis=-1, keepdims=True)
        t = jnp.where(t >= m, -1.0, t)
    thr = t.max(axis=-1, keepdims=True)
    pos_mask = ((metrics >= thr) & (metrics > 0)).astype(align.dtype)
    pos_mask = pos_mask * (pad_bbox_flag > 0)

    # resolve priors matched to multiple gts: keep highest-IoU gt
    fg = pos_mask.sum(axis=1)                          # [b,P]
    multi = fg[:, None, :] > 1
    colmax = overlaps.max(axis=1, keepdims=True)       # [b,1,P]
    is_max = (overlaps == colmax).astype(pos_mask.dtype)
    pos_mask = jnp.where(multi, is_max, pos_mask)
    fg = pos_mask.sum(axis=1)
    fg_bool = fg > 0

    # gather targets via the 0/1 assignment matrix; background -> gt 0
    labels_f = labels.astype(pos_mask.dtype)
    feats = jnp.concatenate([labels_f[..., None], gt_bboxes], axis=-1)   # [b,G,5]
    gath = jnp.einsum('bgp,bgf->bpf', pos_mask, feats)                   # [b,P,5]
    bg = 1.0 - fg_bool.astype(pos_mask.dtype)
    assigned_labels = (gath[..., 0] + labels_f[:, 0:1] * bg).astype(jnp.int32)
    assigned_bboxes = gath[..., 1:5] + gt_bboxes[:, 0, :][:, None, :] * bg[..., None]

    am = align * pos_mask
    pos_align = am.max(axis=-1, keepdims=True)
    pos_over = (overlaps * pos_mask).max(axis=-1, keepdims=True)
    norm = (am * pos_over / (pos_align + EPS)).max(axis=1)[..., None]    # [b,P,1]
    scores = jax.nn.one_hot(assigned_labels, C, dtype=pred_scores.dtype)
    scores = jnp.where(fg_bool[..., None], scores, 0.0)
    assigned_scores = scores * norm

    return assigned_labels, assigned_bboxes, assigned_scores, fg_bool


_pmapped = None


def _get_pmapped():
    global _pmapped
    if _pmapped is None:
        _pmapped = jax.pmap(
            _assign,
            in_axes=(0, 0, None, 0, 0, 0),
            devices=jax.devices()[:NCORES],
        )
    return _pmapped


def kernel(pred_scores, pred_bboxes, priors_points, gt_labels, gt_bboxes, pad_bbox_flag):
    shard = B // NCORES
    sh = lambda x: np.ascontiguousarray(
        np.asarray(x).reshape(NCORES, shard, *np.asarray(x).shape[1:]))
    f = _get_pmapped()
    lab, box, sco, fg = f(
        sh(np.asarray(pred_scores, np.float32)),
        sh(np.asarray(pred_bboxes, np.float32)),
        np.asarray(priors_points, np.float32),
        sh(np.asarray(gt_labels)),
        sh(np.asarray(gt_bboxes, np.float32)),
        sh(np.asarray(pad_bbox_flag, np.float32)),
    )
    lab = np.asarray(lab).reshape(B, P)
    box = np.asarray(box).reshape(B, P, 4)
    sco = np.asarray(sco).reshape(B, P, C)
    fg = np.asarray(fg).reshape(B, P)
    return lab, box, sco, fg
